# revision 8
# baseline (speedup 1.0000x reference)
"""Householder reflection kernel for Trainium2 (Bass/Tile), 8-core SPMD.

Computes z - 2 * v * (v.z)/(v.v) rowwise over [16384, 1024] f32 inputs.
Pure data-parallel: batch dim split evenly across 8 NeuronCores.
"""

import sys

try:
    import concourse  # noqa: F401  (via PYTHONPATH in the normal env)
except ImportError:
    sys.path.append("/opt/trn_rl_repo")

import numpy as np

import concourse.bass as bass
import concourse.tile as tile
from concourse import bacc, mybir
from concourse.alu_op_type import AluOpType
from concourse.bass_utils import run_bass_kernel_spmd

B, L = 16384, 1024
NCORES = 8
SHARD = B // NCORES          # 2048 rows per core
P = 128                      # SBUF partitions
R = 4                        # rows packed per partition (free-dim segments)
ROWS_PER_TILE = P * R        # 512
NT = SHARD // ROWS_PER_TILE  # 4 tiles per core
F32 = mybir.dt.float32


def _build_nc() -> bass.Bass:
    nc = bacc.Bacc("TRN2", target_bir_lowering=False)

    v = nc.declare_dram_parameter("v", [SHARD, L], F32, isOutput=False)
    z = nc.declare_dram_parameter("z", [SHARD, L], F32, isOutput=False)
    o = nc.declare_dram_parameter("o", [SHARD, L], F32, isOutput=True)

    # DRAM row n*P*R + p*R + r  ->  tile n, partition p, segment r.
    # Each tile is a contiguous 2 MiB block of DRAM.
    v3 = v.rearrange("(n p r) d -> n p (r d)", p=P, r=R)
    z3 = z.rearrange("(n p r) d -> n p (r d)", p=P, r=R)
    o3 = o.rearrange("(n p r) d -> n p (r d)", p=P, r=R)

    with tile.TileContext(nc) as tc:
        with (
            tc.tile_pool(name="io", bufs=3) as io_pool,
            tc.tile_pool(name="scratch", bufs=1) as scratch_pool,
            tc.tile_pool(name="stats", bufs=4) as stats_pool,
        ):
            for n in range(NT):
                vt = io_pool.tile([P, R * L], F32, tag="v")
                zt = io_pool.tile([P, R * L], F32, tag="z")
                nc.sync.dma_start(out=vt[:], in_=v3[n])
                nc.sync.dma_start(out=zt[:], in_=z3[n])

                vt3 = vt.rearrange("p (r d) -> p r d", r=R)
                zt3 = zt.rearrange("p (r d) -> p r d", r=R)

                vv = stats_pool.tile([P, R], F32, tag="vv")
                vz = stats_pool.tile([P, R], F32, tag="vz")
                ratio = stats_pool.tile([P, R], F32, tag="ratio")
                sq = scratch_pool.tile([P, R * L], F32, tag="sq")
                sq3 = sq.rearrange("p (r d) -> p r d", r=R)
                prod = scratch_pool.tile([P, R * L], F32, tag="prod")
                prod3 = prod.rearrange("p (r d) -> p r d", r=R)

                for r in range(R):
                    # ACT engine: vv[:, r] = sum(v*v) over the row
                    nc.scalar.activation(
                        out=sq3[:, r, :],
                        in_=vt3[:, r, :],
                        func=mybir.ActivationFunctionType.Square,
                        accum_out=vv[:, r : r + 1],
                    )
                    # DVE: vz[:, r] = sum(-2 * v * z) over the row
                    nc.vector.scalar_tensor_tensor(
                        out=prod3[:, r, :],
                        in0=vt3[:, r, :],
                        scalar=-2.0,
                        in1=zt3[:, r, :],
                        op0=AluOpType.mult,
                        op1=AluOpType.mult,
                        accum_out=vz[:, r : r + 1],
                    )

                # ratio = (-2 * v.z) / (v.v), per row
                rvv = stats_pool.tile([P, R], F32, tag="rvv")
                nc.vector.reciprocal(rvv[:], vv[:])
                nc.vector.tensor_mul(ratio[:], vz[:], rvv[:])

                ot = io_pool.tile([P, R * L], F32, tag="o")
                ot3 = ot.rearrange("p (r d) -> p r d", r=R)
                for r in range(R):
                    # out = (v * ratio) + z  ==  z - 2 v (v.z)/(v.v)
                    nc.vector.scalar_tensor_tensor(
                        out=ot3[:, r, :],
                        in0=vt3[:, r, :],
                        scalar=ratio[:, r : r + 1],
                        in1=zt3[:, r, :],
                        op0=AluOpType.mult,
                        op1=AluOpType.add,
                    )

                nc.sync.dma_start(out=o3[n], in_=ot[:])

    nc.compile()
    return nc


_NC_CACHE = None


def _get_nc() -> bass.Bass:
    global _NC_CACHE
    if _NC_CACHE is None:
        _NC_CACHE = _build_nc()
    return _NC_CACHE


def run_sharded(v: np.ndarray, z: np.ndarray, **spmd_kwargs):
    """Shard inputs over cores, run, return (full_output, BassKernelResults)."""
    nc = _get_nc()
    v = np.ascontiguousarray(v, dtype=np.float32)
    z = np.ascontiguousarray(z, dtype=np.float32)
    in_maps = [
        {
            "v": v[i * SHARD : (i + 1) * SHARD],
            "z": z[i * SHARD : (i + 1) * SHARD],
        }
        for i in range(NCORES)
    ]
    res = run_bass_kernel_spmd(nc, in_maps, list(range(NCORES)), **spmd_kwargs)
    out = np.concatenate([np.asarray(r["o"]) for r in res.results], axis=0)
    return out, res


def kernel(v: np.ndarray, z: np.ndarray) -> np.ndarray:
    out, _ = run_sharded(v, z)
    return out.astype(np.float32)
